# revision 27
# baseline (speedup 1.0000x reference)
"""Additive (Bahdanau) attention on 8 Trainium2 NeuronCores.

Problem shapes (hardcoded): query [2,1024,256], key [2,1024,256],
Wa_w/Wb_w [256,128], Wa_b/Wb_b [128], v_w [128].  Output [2,1024,256].

  a = q @ Wa + Wa_b                  [B,N,H]
  b = k @ Wb + Wb_b                  [B,M,H]
  s[b,n,m] = sum_h v_h tanh(a[b,n,h] + b[b,m,h])
  out = softmax_m(s) @ key           [B,N,D]

Sharding: 8 cores = B(2) x n-halves(2) x m-halves(2).  Each core: 512
queries x 512 keys; each core emits unnormalized exp-score sums
(out_u [512, 256] plus rowsum column); the host adds the two m-halves
and divides (exact softmax merge).

Algorithm: tanh(s) ~ sum_j beta_j sin(om_j s), om_j = j*pi/L (weighted
LS fit; end-to-end rel err ~5e-3 at J=7).  The sine addition theorem
factorizes sin(om_j(a+b)) = sin(om_j a)cos(om_j b) + cos(om_j a)sin(om_j b),
so the [N,M,H] tanh tensor never exists: scores become 2J fp16 matmuls
over the h-contraction per m-block.  Per core:

  aT[h,n] = Wa^T qT, bT[h,m] = Wb^T kT   (PE, fp32r, 1 cyc/row)
  ACT Sin seeds: sin(d x) and sin(d x/2) (all args <= 1.8 rad; the HW
  sin table is only valid on [-pi, pi]); cos via half-angle
  cos(dx) = 1-2sin^2(dx/2) on DVE; harmonics j>=2 via Chebyshev
  recurrence s_j = 2cos(dx)*s_{j-1} - s_{j-2} on DVE in fp16;
  scoresT[m,n] accumulated in PSUM (one bank per m-block) from fp16
  matmuls: lhsT = b-feature [h, m-block], rhs = beta_j v (.) a-feature;
  the beta_j*v folds run on ACT (Copy w/ per-partition scale) for the
  last harmonics and DVE for the early ones (engine balance);
  exp on ACT (scores bounded, no max-shift), fp16;
  out_u[n, d|1] = sum_m exT[m,n] * [key_f16 | 1]  (fp16 matmuls).
"""

import numpy as np

import concourse.bass as bass
import concourse.tile as tile
from concourse import bacc, mybir
from concourse import bass_utils

F32 = mybir.dt.float32
F32R = mybir.dt.float32r
F16 = mybir.dt.float16

B, N, M, D, H = 2, 1024, 1024, 256, 128
NCORES = 8
NQ = 512           # queries per core
NM = 512           # keys per core
NB = NQ // 128     # 4 n-blocks
MB = NM // 128     # 4 m-blocks

J = 7
L = 8.4
DELTA = float(np.pi / L)


def _fit_beta():
    om = np.arange(1, J + 1) * DELTA
    x = np.linspace(0, 8.6, 6000)
    w2 = np.exp(-0.5 * (x / 1.41) ** 2) + 10 ** -2.5
    A = np.sin(np.outer(x, om))
    beta = np.linalg.solve((A * w2[:, None]).T @ A, (A * w2[:, None]).T @ np.tanh(x))
    return beta


BETA = _fit_beta()


def build_nc(reps: int = 1, **opts):
    nc = bacc.Bacc(
        "TRN2",
        target_bir_lowering=False,
        debug=False,
        enable_asserts=False,
        num_devices=NCORES,
    )
    MMDT = F32R if opts.get("f32r", True) else F32
    qT_d = nc.dram_tensor("qT", [D, NQ], MMDT, kind="ExternalInput").ap()
    kT_d = nc.dram_tensor("kT", [D, NM], MMDT, kind="ExternalInput").ap()
    kf_d = nc.dram_tensor("kf", [NM, D], F16, kind="ExternalInput").ap()
    wa_d = nc.dram_tensor("wa", [D, H], MMDT, kind="ExternalInput").ap()
    wb_d = nc.dram_tensor("wb", [D, H], MMDT, kind="ExternalInput").ap()
    sbias_d = nc.dram_tensor("sbias", [H, 1], F32, kind="ExternalInput").ap()
    hbias_d = nc.dram_tensor("hbias", [H, 1], F32, kind="ExternalInput").ap()
    vbeta_d = nc.dram_tensor("vbeta", [H, J], F32, kind="ExternalInput").ap()
    out_d = nc.dram_tensor("out", [NQ, D + 1], F32, kind="ExternalOutput").ap()

    with tile.TileContext(nc) as tc:
        _build_body(tc, qT_d, kT_d, kf_d, wa_d, wb_d, sbias_d, hbias_d,
                    vbeta_d, out_d, reps, **opts)
    nc.compile()
    return nc


def _build_body(tc, qT_d, kT_d, kf_d, wa_d, wb_d, sbias_d, hbias_d, vbeta_d,
                out_d, reps, f32r=True, wbufs=2, fold_pool=0, fold_act=0,
                sq_act=False):
    nc = tc.nc
    KT = D // 128  # 2 contraction tiles over d
    Sin = mybir.ActivationFunctionType.Sin
    Exp = mybir.ActivationFunctionType.Exp
    MMDT = F32R if f32r else F32
    MULT = mybir.AluOpType.mult
    ADD = mybir.AluOpType.add

    with (
        tc.tile_pool(name="persist", bufs=1) as pp,
        tc.tile_pool(name="work", bufs=wbufs) as wp,
        tc.tile_pool(name="small", bufs=4) as sp,
    ):
        # ---- static loads ----
        wa_sb, wb_sb, qT_sb, kT_sb = [], [], [], []
        for dt_ in range(KT):
            t = pp.tile([128, H], MMDT, name=f"wa{dt_}")
            nc.sync.dma_start(t[:], wa_d[dt_ * 128:(dt_ + 1) * 128, :])
            wa_sb.append(t)
            t = pp.tile([128, H], MMDT, name=f"wb{dt_}")
            nc.sync.dma_start(t[:], wb_d[dt_ * 128:(dt_ + 1) * 128, :])
            wb_sb.append(t)
            t = pp.tile([128, NQ], MMDT, name=f"qT{dt_}")
            nc.sync.dma_start(t[:], qT_d[dt_ * 128:(dt_ + 1) * 128, :])
            qT_sb.append(t)
            t = pp.tile([128, NM], MMDT, name=f"kT{dt_}")
            nc.sync.dma_start(t[:], kT_d[dt_ * 128:(dt_ + 1) * 128, :])
            kT_sb.append(t)
        sbias_sb = pp.tile([128, 1], F32, name="sbias")
        nc.sync.dma_start(sbias_sb[:], sbias_d[:, :])
        hbias_sb = pp.tile([128, 1], F32, name="hbias")
        nc.sync.dma_start(hbias_sb[:], hbias_d[:, :])
        vbeta_sb = pp.tile([128, J], F32, name="vbeta")
        nc.sync.dma_start(vbeta_sb[:], vbeta_d[:, :])
        kf_sb = []
        for mt in range(MB):
            t = pp.tile([128, D + 1], F16, name=f"kf{mt}")
            nc.sync.dma_start(t[:, :D], kf_d[mt * 128:(mt + 1) * 128, :])
            nc.gpsimd.memset(t[:, D:D + 1], 1.0)
            kf_sb.append(t)
        zero_sb = pp.tile([128, 1], F32, name="zero_sb")
        nc.gpsimd.memset(zero_sb[:], 0.0)

        # per-harmonic state slots (no reuse within a rep)
        NS = J + 1
        sa_sl = [pp.tile([128, NQ], F16, name=f"sa{i}") for i in range(NS)]
        ca_sl = [pp.tile([128, NQ], F16, name=f"ca{i}") for i in range(NS)]
        sb_sl = [pp.tile([128, NM], F16, name=f"sb{i}") for i in range(NS)]
        cb_sl = [pp.tile([128, NM], F16, name=f"cb{i}") for i in range(NS)]
        m2a = pp.tile([128, NQ], F16, name="m2a")
        m2b = pp.tile([128, NM], F16, name="m2b")

        with (
            tc.tile_pool(name="pb_ps", bufs=1, space="PSUM") as pbp,
            tc.tile_pool(name="sc_ps", bufs=1, space="PSUM") as scp,
        ):
            for _ in range(reps):
                # ---- projections (fp32r: 1 cyc/row at free>=256) ----
                ps_a = pbp.tile([128, NQ], F32, name="ps_a")
                ps_b = pbp.tile([128, NM], F32, name="ps_b")
                for dt_ in range(KT):
                    nc.tensor.matmul(
                        ps_a[:], wa_sb[dt_][:], qT_sb[dt_][:],
                        start=(dt_ == 0), stop=(dt_ == KT - 1),
                    )
                for dt_ in range(KT):
                    nc.tensor.matmul(
                        ps_b[:], wb_sb[dt_][:], kT_sb[dt_][:],
                        start=(dt_ == 0), stop=(dt_ == KT - 1),
                    )

                # ---- seeds: sin(d x), sin(d x / 2); cos via half-angle ----
                sa = {1: sa_sl[1]}
                ca = {1: ca_sl[1]}
                sb = {1: sb_sl[1]}
                cb = {1: cb_sl[1]}
                nc.scalar.activation(sa[1][:], ps_a[:], Sin, scale=DELTA,
                                     bias=zero_sb[:])
                ha = wp.tile([128, NQ], F16, name="ha")
                nc.scalar.activation(ha[:], ps_a[:], Sin, scale=DELTA / 2,
                                     bias=zero_sb[:])
                nc.scalar.activation(sb[1][:], ps_b[:], Sin, scale=DELTA,
                                     bias=sbias_sb[:])
                hb = wp.tile([128, NM], F16, name="hb")
                nc.scalar.activation(hb[:], ps_b[:], Sin, scale=DELTA / 2,
                                     bias=hbias_sb[:])
                h2a = wp.tile([128, NQ], F16, name="h2a")
                if sq_act:
                    nc.scalar.square(h2a[:], ha[:])
                else:
                    nc.vector.tensor_mul(h2a[:], ha[:], ha[:])
                nc.vector.tensor_scalar(ca[1][:], h2a[:], -2.0, 1.0, MULT, ADD)
                nc.vector.tensor_scalar(m2a[:], h2a[:], -4.0, 2.0, MULT, ADD)
                h2b = wp.tile([128, NM], F16, name="h2b")
                if sq_act:
                    nc.scalar.square(h2b[:], hb[:])
                else:
                    nc.vector.tensor_mul(h2b[:], hb[:], hb[:])
                nc.vector.tensor_scalar(cb[1][:], h2b[:], -2.0, 1.0, MULT, ADD)
                nc.vector.tensor_scalar(m2b[:], h2b[:], -4.0, 2.0, MULT, ADD)

                def recur(j, s, c, s_sl, c_sl, m2, w, tag):
                    ssj, csj = s_sl[j], c_sl[j]
                    if j == 2:
                        nc.vector.tensor_mul(ssj[:], m2[:], s[1][:])
                        t = wp.tile([128, w], F16, name=f"rc{tag}")
                        nc.vector.tensor_mul(t[:], m2[:], c[1][:])
                        nc.vector.tensor_scalar_add(csj[:], t[:], -1.0)
                    else:
                        t1 = wp.tile([128, w], F16, name=f"rs{tag}")
                        nc.vector.tensor_mul(t1[:], m2[:], s[j - 1][:])
                        nc.vector.tensor_sub(ssj[:], t1[:], s[j - 2][:])
                        t2 = wp.tile([128, w], F16, name=f"rc{tag}")
                        nc.vector.tensor_mul(t2[:], m2[:], c[j - 1][:])
                        nc.vector.tensor_sub(csj[:], t2[:], c[j - 2][:])
                    s[j], c[j] = ssj, csj

                # scores: one PSUM bank per m-block, [m(128), n(512)]
                sc_t = [scp.tile([128, NQ], F32, name=f"sc{i}")
                        for i in range(MB)]
                exT = wp.tile([128, MB * NQ], F16, name="exT")

                for j in range(1, J + 1):
                    if j >= 2:
                        recur(j, sa, ca, sa_sl, ca_sl, m2a, NQ, "a")
                        recur(j, sb, cb, sb_sl, cb_sl, m2b, NM, "b")
                    fa_s = wp.tile([128, NQ], F16, name="fas")
                    fa_c = wp.tile([128, NQ], F16, name="fac")
                    if j > J - fold_act:
                        nc.scalar.mul(fa_s[:], sa[j][:], vbeta_sb[:, j - 1:j])
                        nc.scalar.mul(fa_c[:], ca[j][:], vbeta_sb[:, j - 1:j])
                    else:
                        eng = nc.gpsimd if j <= fold_pool else nc.vector
                        eng.tensor_scalar_mul(fa_s[:], sa[j][:],
                                              vbeta_sb[:, j - 1:j])
                        eng.tensor_scalar_mul(fa_c[:], ca[j][:],
                                              vbeta_sb[:, j - 1:j])
                    for mb in range(MB):
                        bsl = slice(mb * 128, (mb + 1) * 128)
                        nc.tensor.matmul(
                            sc_t[mb][:], cb[j][:, bsl], fa_s[:],
                            start=(j == 1), stop=False,
                        )
                        nc.tensor.matmul(
                            sc_t[mb][:], sb[j][:, bsl], fa_c[:],
                            start=False, stop=(j == J),
                        )

                # exp (scores bounded: no max-shift needed)
                for mb in range(MB):
                    nc.scalar.activation(
                        exT[:, mb * NQ:(mb + 1) * NQ], sc_t[mb][:], Exp)

                # out_u[n, d|1] = sum_m exT[m, n] [kf | 1]
                for nbk in range(NB):
                    po = pbp.tile([128, 512], F32, name="po")
                    for mb in range(MB):
                        nc.tensor.matmul(
                            po[:, :D + 1],
                            exT[:, mb * NQ + nbk * 128: mb * NQ + nbk * 128 + 128],
                            kf_sb[mb][:],
                            start=(mb == 0), stop=(mb == MB - 1),
                        )
                    osb = sp.tile([128, D + 1], F32, name="osb")
                    nc.vector.tensor_copy(osb[:], po[:, :D + 1])
                    nc.sync.dma_start(
                        out_d[nbk * 128:(nbk + 1) * 128, :], osb[:])


def _in_maps(inputs):
    q = np.asarray(inputs["query"], dtype=np.float32)
    k = np.asarray(inputs["key"], dtype=np.float32)
    wa = np.ascontiguousarray(np.asarray(inputs["Wa_w"], dtype=np.float32))
    wb = np.ascontiguousarray(np.asarray(inputs["Wb_w"], dtype=np.float32))
    bias = (np.asarray(inputs["Wa_b"], dtype=np.float32)
            + np.asarray(inputs["Wb_b"], dtype=np.float32))
    v = np.asarray(inputs["v_w"], dtype=np.float32)
    sbias = (DELTA * bias).reshape(H, 1).astype(np.float32)
    hbias = (DELTA / 2 * bias).reshape(H, 1).astype(np.float32)
    vbeta = (v[:, None] * BETA[None, :]).astype(np.float32)
    maps = []
    for c in range(NCORES):
        b, nh, mh = c >> 2, (c >> 1) & 1, c & 1
        qs = q[b, nh * NQ:(nh + 1) * NQ, :]
        ks = k[b, mh * NM:(mh + 1) * NM, :]
        maps.append({
            "qT": np.ascontiguousarray(qs.T),
            "kT": np.ascontiguousarray(ks.T),
            "kf": np.ascontiguousarray(ks.astype(np.float16)),
            "wa": wa,
            "wb": wb,
            "sbias": sbias,
            "hbias": hbias,
            "vbeta": vbeta,
        })
    return maps


def _gather(results):
    out = np.empty((B, N, D), dtype=np.float32)
    for b in range(B):
        for nh in range(2):
            u0 = results[b * 4 + nh * 2 + 0]["out"]
            u1 = results[b * 4 + nh * 2 + 1]["out"]
            num = u0[:, :D] + u1[:, :D]
            den = u0[:, D] + u1[:, D]
            out[b, nh * NQ:(nh + 1) * NQ, :] = num / den[:, None]
    return out


_NC_CACHE = {}

BEST_OPTS = dict(fold_act=6, sq_act=True, wbufs=3)


def _get_nc(reps=1):
    if reps not in _NC_CACHE:
        _NC_CACHE[reps] = build_nc(reps, **BEST_OPTS)
    return _NC_CACHE[reps]


def kernel(**inputs):
    nc = _get_nc(1)
    res = bass_utils.run_bass_kernel_spmd(
        nc, _in_maps(inputs), core_ids=list(range(NCORES))
    )
    return _gather(res.results)


# revision 29
# speedup vs baseline: 1.1482x; 1.1482x over previous
"""Additive (Bahdanau) attention on 8 Trainium2 NeuronCores.

Problem shapes (hardcoded): query [2,1024,256], key [2,1024,256],
Wa_w/Wb_w [256,128], Wa_b/Wb_b [128], v_w [128].  Output [2,1024,256].

  a = q @ Wa + Wa_b                  [B,N,H]
  b = k @ Wb + Wb_b                  [B,M,H]
  s[b,n,m] = sum_h v_h tanh(a[b,n,h] + b[b,m,h])
  out = softmax_m(s) @ key           [B,N,D]

Sharding: 8 cores = B(2) x n-halves(2) x m-halves(2).  Each core: 512
queries x 512 keys; each core emits unnormalized exp-score sums
(out_u [512, 256] plus rowsum column); the host adds the two m-halves
and divides (exact softmax merge).

Algorithm: tanh(s) ~ sum_j beta_j sin(om_j s), om_j = j*pi/L (weighted
LS fit; end-to-end rel err ~5e-3 at J=7).  The sine addition theorem
factorizes sin(om_j(a+b)) = sin(om_j a)cos(om_j b) + cos(om_j a)sin(om_j b),
so the [N,M,H] tanh tensor never exists: scores become 2J fp16 matmuls
over the h-contraction per m-block.  Per core:

  aT[h,n] = Wa^T qT, bT[h,m] = Wb^T kT   (PE, fp32r, 1 cyc/row)
  ACT Sin seeds: sin(d x) and sin(d x/2) (all args <= 1.8 rad; the HW
  sin table is only valid on [-pi, pi]); cos via half-angle
  cos(dx) = 1-2sin^2(dx/2) on DVE; harmonics j>=2 via Chebyshev
  recurrence s_j = 2cos(dx)*s_{j-1} - s_{j-2} on DVE in fp16;
  scoresT[m,n] accumulated in PSUM (one bank per m-block) from fp16
  matmuls: lhsT = b-feature [h, m-block], rhs = beta_j v (.) a-feature;
  the beta_j*v folds run on ACT (Copy w/ per-partition scale) for the
  last harmonics and DVE for the early ones (engine balance);
  exp on ACT (scores bounded, no max-shift), fp16;
  out_u[n, d|1] = sum_m exT[m,n] * [key_f16 | 1]  (fp16 matmuls).
"""

import numpy as np

import concourse.bass as bass
import concourse.tile as tile
from concourse import bacc, mybir
from concourse import bass_utils

F32 = mybir.dt.float32
F32R = mybir.dt.float32r
F16 = mybir.dt.float16

B, N, M, D, H = 2, 1024, 1024, 256, 128
NCORES = 8
NQ = 512           # queries per core
NM = 512           # keys per core
NB = NQ // 128     # 4 n-blocks
MB = NM // 128     # 4 m-blocks

J = 7
L = 8.4
DELTA = float(np.pi / L)


def _fit_beta():
    om = np.arange(1, J + 1) * DELTA
    x = np.linspace(0, 8.6, 6000)
    w2 = np.exp(-0.5 * (x / 1.41) ** 2) + 10 ** -2.5
    A = np.sin(np.outer(x, om))
    beta = np.linalg.solve((A * w2[:, None]).T @ A, (A * w2[:, None]).T @ np.tanh(x))
    return beta


BETA = _fit_beta()


def build_nc(reps: int = 1, **opts):
    nc = bacc.Bacc(
        "TRN2",
        target_bir_lowering=False,
        debug=False,
        enable_asserts=False,
        num_devices=NCORES,
    )
    MMDT = F32R if opts.get("f32r", True) else F32
    qT_d = nc.dram_tensor("qT", [D, NQ], MMDT, kind="ExternalInput").ap()
    kT_d = nc.dram_tensor("kT", [D, NM], MMDT, kind="ExternalInput").ap()
    kf_d = nc.dram_tensor("kf", [NM, D], F16, kind="ExternalInput").ap()
    wa_d = nc.dram_tensor("wa", [D, H], MMDT, kind="ExternalInput").ap()
    wb_d = nc.dram_tensor("wb", [D, H], MMDT, kind="ExternalInput").ap()
    sbias_d = nc.dram_tensor("sbias", [H, 1], F32, kind="ExternalInput").ap()
    hbias_d = nc.dram_tensor("hbias", [H, 1], F32, kind="ExternalInput").ap()
    vbeta_d = nc.dram_tensor("vbeta", [H, J], F32, kind="ExternalInput").ap()
    out_d = nc.dram_tensor("out", [NQ, D + 1], F32, kind="ExternalOutput").ap()

    with tile.TileContext(nc) as tc:
        _build_body(tc, qT_d, kT_d, kf_d, wa_d, wb_d, sbias_d, hbias_d,
                    vbeta_d, out_d, reps, **opts)
    nc.compile()
    return nc


def _build_body(tc, qT_d, kT_d, kf_d, wa_d, wb_d, sbias_d, hbias_d, vbeta_d,
                out_d, reps, f32r=True, wbufs=2, fold_pool=0, fold_act=0,
                sq_act=False):
    nc = tc.nc
    KT = D // 128  # 2 contraction tiles over d
    Sin = mybir.ActivationFunctionType.Sin
    Exp = mybir.ActivationFunctionType.Exp
    MMDT = F32R if f32r else F32
    MULT = mybir.AluOpType.mult
    ADD = mybir.AluOpType.add

    with (
        tc.tile_pool(name="persist", bufs=1) as pp,
        tc.tile_pool(name="work", bufs=wbufs) as wp,
        tc.tile_pool(name="small", bufs=4) as sp,
    ):
        # ---- static loads ----
        wa_sb, wb_sb, qT_sb, kT_sb = [], [], [], []
        for dt_ in range(KT):
            t = pp.tile([128, H], MMDT, name=f"wa{dt_}")
            nc.sync.dma_start(t[:], wa_d[dt_ * 128:(dt_ + 1) * 128, :])
            wa_sb.append(t)
            t = pp.tile([128, H], MMDT, name=f"wb{dt_}")
            nc.sync.dma_start(t[:], wb_d[dt_ * 128:(dt_ + 1) * 128, :])
            wb_sb.append(t)
            t = pp.tile([128, NQ], MMDT, name=f"qT{dt_}")
            nc.sync.dma_start(t[:], qT_d[dt_ * 128:(dt_ + 1) * 128, :])
            qT_sb.append(t)
            t = pp.tile([128, NM], MMDT, name=f"kT{dt_}")
            nc.sync.dma_start(t[:], kT_d[dt_ * 128:(dt_ + 1) * 128, :])
            kT_sb.append(t)
        sbias_sb = pp.tile([128, 1], F32, name="sbias")
        nc.sync.dma_start(sbias_sb[:], sbias_d[:, :])
        hbias_sb = pp.tile([128, 1], F32, name="hbias")
        nc.sync.dma_start(hbias_sb[:], hbias_d[:, :])
        vbeta_sb = pp.tile([128, J], F32, name="vbeta")
        nc.sync.dma_start(vbeta_sb[:], vbeta_d[:, :])
        kf_sb = []
        for mt in range(MB):
            t = pp.tile([128, D + 1], F16, name=f"kf{mt}")
            nc.sync.dma_start(t[:, :D], kf_d[mt * 128:(mt + 1) * 128, :])
            nc.gpsimd.memset(t[:, D:D + 1], 1.0)
            kf_sb.append(t)
        zero_sb = pp.tile([128, 1], F32, name="zero_sb")
        nc.gpsimd.memset(zero_sb[:], 0.0)

        # per-harmonic state slots, sin|cos paired in one tile (halves the
        # DVE instruction count of the recurrence)
        NS = J + 1
        pa_sl = [pp.tile([128, 2 * NQ], F16, name=f"pa{i}") for i in range(NS)]
        pb_sl = [pp.tile([128, 2 * NM], F16, name=f"pb{i}") for i in range(NS)]
        m2a = pp.tile([128, 2 * NQ], F16, name="m2a")
        m2b = pp.tile([128, 2 * NM], F16, name="m2b")

        with (
            tc.tile_pool(name="pb_ps", bufs=1, space="PSUM") as pbp,
            tc.tile_pool(name="sc_ps", bufs=1, space="PSUM") as scp,
        ):
            for _ in range(reps):
                # ---- projections (fp32r: 1 cyc/row at free>=256) ----
                ps_a = pbp.tile([128, NQ], F32, name="ps_a")
                ps_b = pbp.tile([128, NM], F32, name="ps_b")
                for dt_ in range(KT):
                    nc.tensor.matmul(
                        ps_a[:], wa_sb[dt_][:], qT_sb[dt_][:],
                        start=(dt_ == 0), stop=(dt_ == KT - 1),
                    )
                for dt_ in range(KT):
                    nc.tensor.matmul(
                        ps_b[:], wb_sb[dt_][:], kT_sb[dt_][:],
                        start=(dt_ == 0), stop=(dt_ == KT - 1),
                    )

                # ---- seeds: sin(d x), sin(d x / 2); cos via half-angle
                # (pair layout: [:, :W] = sin, [:, W:] = cos) ----
                pa = {1: pa_sl[1]}
                pb = {1: pb_sl[1]}
                nc.scalar.activation(pa[1][:, :NQ], ps_a[:], Sin, scale=DELTA,
                                     bias=zero_sb[:])
                ha = wp.tile([128, NQ], F16, name="ha")
                nc.scalar.activation(ha[:], ps_a[:], Sin, scale=DELTA / 2,
                                     bias=zero_sb[:])
                nc.scalar.activation(pb[1][:, :NM], ps_b[:], Sin, scale=DELTA,
                                     bias=sbias_sb[:])
                hb = wp.tile([128, NM], F16, name="hb")
                nc.scalar.activation(hb[:], ps_b[:], Sin, scale=DELTA / 2,
                                     bias=hbias_sb[:])
                h2a = wp.tile([128, NQ], F16, name="h2a")
                if sq_act:
                    nc.scalar.square(h2a[:], ha[:])
                else:
                    nc.vector.tensor_mul(h2a[:], ha[:], ha[:])
                nc.vector.tensor_scalar(pa[1][:, NQ:], h2a[:], -2.0, 1.0,
                                        MULT, ADD)
                nc.vector.tensor_scalar(m2a[:, :NQ], h2a[:], -4.0, 2.0,
                                        MULT, ADD)
                nc.vector.tensor_scalar(m2a[:, NQ:], h2a[:], -4.0, 2.0,
                                        MULT, ADD)
                h2b = wp.tile([128, NM], F16, name="h2b")
                if sq_act:
                    nc.scalar.square(h2b[:], hb[:])
                else:
                    nc.vector.tensor_mul(h2b[:], hb[:], hb[:])
                nc.vector.tensor_scalar(pb[1][:, NM:], h2b[:], -2.0, 1.0,
                                        MULT, ADD)
                nc.vector.tensor_scalar(m2b[:, :NM], h2b[:], -4.0, 2.0,
                                        MULT, ADD)
                nc.vector.tensor_scalar(m2b[:, NM:], h2b[:], -4.0, 2.0,
                                        MULT, ADD)

                def recur(j, p, p_sl, m2, w, tag):
                    pj = p_sl[j]
                    t = wp.tile([128, 2 * w], F16, name=f"r{tag}")
                    nc.vector.tensor_mul(t[:], m2[:], p[j - 1][:])
                    if j == 2:
                        # s0 = 0, c0 = 1
                        nc.vector.tensor_copy(pj[:, :w], t[:, :w])
                        nc.vector.tensor_scalar_add(pj[:, w:], t[:, w:], -1.0)
                    else:
                        nc.vector.tensor_sub(pj[:], t[:], p[j - 2][:])
                    p[j] = pj

                # scores: one PSUM bank per m-block, [m(128), n(512)]
                sc_t = [scp.tile([128, NQ], F32, name=f"sc{i}")
                        for i in range(MB)]
                exT = wp.tile([128, MB * NQ], F16, name="exT")

                for j in range(1, J + 1):
                    if j >= 2:
                        recur(j, pa, pa_sl, m2a, NQ, "a")
                        recur(j, pb, pb_sl, m2b, NM, "b")
                    fa = wp.tile([128, 2 * NQ], F16, name="fa")
                    if j > J - fold_act:
                        nc.scalar.mul(fa[:], pa[j][:], vbeta_sb[:, j - 1:j])
                    else:
                        nc.vector.tensor_scalar_mul(fa[:], pa[j][:],
                                                    vbeta_sb[:, j - 1:j])
                    for mb in range(MB):
                        bsl = slice(mb * 128, (mb + 1) * 128)
                        csl = slice(NM + mb * 128, NM + (mb + 1) * 128)
                        nc.tensor.matmul(
                            sc_t[mb][:], pb[j][:, csl], fa[:, :NQ],
                            start=(j == 1), stop=False,
                        )
                        nc.tensor.matmul(
                            sc_t[mb][:], pb[j][:, bsl], fa[:, NQ:],
                            start=False, stop=(j == J),
                        )

                # exp (scores bounded: no max-shift needed)
                for mb in range(MB):
                    nc.scalar.activation(
                        exT[:, mb * NQ:(mb + 1) * NQ], sc_t[mb][:], Exp)

                # out_u[n, d|1] = sum_m exT[m, n] [kf | 1]
                for nbk in range(NB):
                    po = pbp.tile([128, 512], F32, name="po")
                    for mb in range(MB):
                        nc.tensor.matmul(
                            po[:, :D + 1],
                            exT[:, mb * NQ + nbk * 128: mb * NQ + nbk * 128 + 128],
                            kf_sb[mb][:],
                            start=(mb == 0), stop=(mb == MB - 1),
                        )
                    osb = sp.tile([128, D + 1], F32, name="osb")
                    nc.vector.tensor_copy(osb[:], po[:, :D + 1])
                    nc.sync.dma_start(
                        out_d[nbk * 128:(nbk + 1) * 128, :], osb[:])


def _in_maps(inputs):
    q = np.asarray(inputs["query"], dtype=np.float32)
    k = np.asarray(inputs["key"], dtype=np.float32)
    wa = np.ascontiguousarray(np.asarray(inputs["Wa_w"], dtype=np.float32))
    wb = np.ascontiguousarray(np.asarray(inputs["Wb_w"], dtype=np.float32))
    bias = (np.asarray(inputs["Wa_b"], dtype=np.float32)
            + np.asarray(inputs["Wb_b"], dtype=np.float32))
    v = np.asarray(inputs["v_w"], dtype=np.float32)
    sbias = (DELTA * bias).reshape(H, 1).astype(np.float32)
    hbias = (DELTA / 2 * bias).reshape(H, 1).astype(np.float32)
    vbeta = (v[:, None] * BETA[None, :]).astype(np.float32)
    maps = []
    for c in range(NCORES):
        b, nh, mh = c >> 2, (c >> 1) & 1, c & 1
        qs = q[b, nh * NQ:(nh + 1) * NQ, :]
        ks = k[b, mh * NM:(mh + 1) * NM, :]
        maps.append({
            "qT": np.ascontiguousarray(qs.T),
            "kT": np.ascontiguousarray(ks.T),
            "kf": np.ascontiguousarray(ks.astype(np.float16)),
            "wa": wa,
            "wb": wb,
            "sbias": sbias,
            "hbias": hbias,
            "vbeta": vbeta,
        })
    return maps


def _gather(results):
    out = np.empty((B, N, D), dtype=np.float32)
    for b in range(B):
        for nh in range(2):
            u0 = results[b * 4 + nh * 2 + 0]["out"]
            u1 = results[b * 4 + nh * 2 + 1]["out"]
            num = u0[:, :D] + u1[:, :D]
            den = u0[:, D] + u1[:, D]
            out[b, nh * NQ:(nh + 1) * NQ, :] = num / den[:, None]
    return out


_NC_CACHE = {}

BEST_OPTS = dict(fold_act=6, sq_act=True, wbufs=3)


def _get_nc(reps=1):
    if reps not in _NC_CACHE:
        _NC_CACHE[reps] = build_nc(reps, **BEST_OPTS)
    return _NC_CACHE[reps]


def kernel(**inputs):
    nc = _get_nc(1)
    res = bass_utils.run_bass_kernel_spmd(
        nc, _in_maps(inputs), core_ids=list(range(NCORES))
    )
    return _gather(res.results)


# revision 30
# speedup vs baseline: 1.1742x; 1.0226x over previous
"""Additive (Bahdanau) attention on 8 Trainium2 NeuronCores.

Problem shapes (hardcoded): query [2,1024,256], key [2,1024,256],
Wa_w/Wb_w [256,128], Wa_b/Wb_b [128], v_w [128].  Output [2,1024,256].

  a = q @ Wa + Wa_b                  [B,N,H]
  b = k @ Wb + Wb_b                  [B,M,H]
  s[b,n,m] = sum_h v_h tanh(a[b,n,h] + b[b,m,h])
  out = softmax_m(s) @ key           [B,N,D]

Sharding: 8 cores = B(2) x n-halves(2) x m-halves(2).  Each core: 512
queries x 512 keys; each core emits unnormalized exp-score sums
(out_u [512, 256] plus rowsum column); the host adds the two m-halves
and divides (exact softmax merge).

Algorithm: tanh(s) ~ sum_j beta_j sin(om_j s), om_j = j*pi/L (weighted
LS fit; end-to-end rel err ~5e-3 at J=7).  The sine addition theorem
factorizes sin(om_j(a+b)) = sin(om_j a)cos(om_j b) + cos(om_j a)sin(om_j b),
so the [N,M,H] tanh tensor never exists: scores become 2J fp16 matmuls
over the h-contraction per m-block.  Per core:

  aT[h,n] = Wa^T qT, bT[h,m] = Wb^T kT   (PE, fp32r, 1 cyc/row)
  ACT Sin seeds: sin(d x) and sin(d x/2) (all args <= 1.8 rad; the HW
  sin table is only valid on [-pi, pi]); cos via half-angle
  cos(dx) = 1-2sin^2(dx/2) on DVE; harmonics j>=2 via Chebyshev
  recurrence s_j = 2cos(dx)*s_{j-1} - s_{j-2} on DVE in fp16;
  scoresT[m,n] accumulated in PSUM (one bank per m-block) from fp16
  matmuls: lhsT = b-feature [h, m-block], rhs = beta_j v (.) a-feature;
  the beta_j*v folds run on ACT (Copy w/ per-partition scale) for the
  last harmonics and DVE for the early ones (engine balance);
  exp on ACT (scores bounded, no max-shift), fp16;
  out_u[n, d|1] = sum_m exT[m,n] * [key_f16 | 1]  (fp16 matmuls).
"""

import numpy as np

import concourse.bass as bass
import concourse.tile as tile
from concourse import bacc, mybir
from concourse import bass_utils

F32 = mybir.dt.float32
F32R = mybir.dt.float32r
F16 = mybir.dt.float16

B, N, M, D, H = 2, 1024, 1024, 256, 128
NCORES = 8
NQ = 512           # queries per core
NM = 512           # keys per core
NB = NQ // 128     # 4 n-blocks
MB = NM // 128     # 4 m-blocks

J = 7
L = 8.4
DELTA = float(np.pi / L)


def _fit_beta():
    om = np.arange(1, J + 1) * DELTA
    x = np.linspace(0, 8.6, 6000)
    w2 = np.exp(-0.5 * (x / 1.41) ** 2) + 10 ** -2.5
    A = np.sin(np.outer(x, om))
    beta = np.linalg.solve((A * w2[:, None]).T @ A, (A * w2[:, None]).T @ np.tanh(x))
    return beta


BETA = _fit_beta()


def build_nc(reps: int = 1, **opts):
    nc = bacc.Bacc(
        "TRN2",
        target_bir_lowering=False,
        debug=False,
        enable_asserts=False,
        num_devices=NCORES,
    )
    MMDT = F32R if opts.get("f32r", True) else F32
    qT_d = nc.dram_tensor("qT", [D, NQ], MMDT, kind="ExternalInput").ap()
    kT_d = nc.dram_tensor("kT", [D, NM], MMDT, kind="ExternalInput").ap()
    kf_d = nc.dram_tensor("kf", [NM, D], F16, kind="ExternalInput").ap()
    wa_d = nc.dram_tensor("wa", [D, H], MMDT, kind="ExternalInput").ap()
    wb_d = nc.dram_tensor("wb", [D, H], MMDT, kind="ExternalInput").ap()
    sbias_d = nc.dram_tensor("sbias", [H, 1], F32, kind="ExternalInput").ap()
    hbias_d = nc.dram_tensor("hbias", [H, 1], F32, kind="ExternalInput").ap()
    vbeta_d = nc.dram_tensor("vbeta", [H, J], F32, kind="ExternalInput").ap()
    out_d = nc.dram_tensor("out", [NQ, D + 1], F32, kind="ExternalOutput").ap()

    with tile.TileContext(nc) as tc:
        _build_body(tc, qT_d, kT_d, kf_d, wa_d, wb_d, sbias_d, hbias_d,
                    vbeta_d, out_d, reps, **opts)
    nc.compile()
    return nc


def _build_body(tc, qT_d, kT_d, kf_d, wa_d, wb_d, sbias_d, hbias_d, vbeta_d,
                out_d, reps, f32r=True, wbufs=2, fold_pool=0, fold_act=0,
                sq_act=False):
    nc = tc.nc
    KT = D // 128  # 2 contraction tiles over d
    Sin = mybir.ActivationFunctionType.Sin
    Exp = mybir.ActivationFunctionType.Exp
    MMDT = F32R if f32r else F32
    MULT = mybir.AluOpType.mult
    ADD = mybir.AluOpType.add

    with (
        tc.tile_pool(name="persist", bufs=1) as pp,
        tc.tile_pool(name="work", bufs=wbufs) as wp,
        tc.tile_pool(name="small", bufs=4) as sp,
    ):
        # ---- static loads ----
        wa_sb, wb_sb, qT_sb, kT_sb = [], [], [], []
        for dt_ in range(KT):
            t = pp.tile([128, H], MMDT, name=f"wa{dt_}")
            nc.sync.dma_start(t[:], wa_d[dt_ * 128:(dt_ + 1) * 128, :])
            wa_sb.append(t)
            t = pp.tile([128, H], MMDT, name=f"wb{dt_}")
            nc.sync.dma_start(t[:], wb_d[dt_ * 128:(dt_ + 1) * 128, :])
            wb_sb.append(t)
            t = pp.tile([128, NQ], MMDT, name=f"qT{dt_}")
            nc.sync.dma_start(t[:], qT_d[dt_ * 128:(dt_ + 1) * 128, :])
            qT_sb.append(t)
            t = pp.tile([128, NM], MMDT, name=f"kT{dt_}")
            nc.sync.dma_start(t[:], kT_d[dt_ * 128:(dt_ + 1) * 128, :])
            kT_sb.append(t)
        sbias_sb = pp.tile([128, 1], F32, name="sbias")
        nc.sync.dma_start(sbias_sb[:], sbias_d[:, :])
        hbias_sb = pp.tile([128, 1], F32, name="hbias")
        nc.sync.dma_start(hbias_sb[:], hbias_d[:, :])
        vbeta_sb = pp.tile([128, J], F32, name="vbeta")
        nc.sync.dma_start(vbeta_sb[:], vbeta_d[:, :])
        kf_sb = []
        for mt in range(MB):
            t = pp.tile([128, D + 1], F16, name=f"kf{mt}")
            nc.sync.dma_start(t[:, :D], kf_d[mt * 128:(mt + 1) * 128, :])
            nc.gpsimd.memset(t[:, D:D + 1], 1.0)
            kf_sb.append(t)
        zero_sb = pp.tile([128, 1], F32, name="zero_sb")
        nc.gpsimd.memset(zero_sb[:], 0.0)

        # per-harmonic state slots, sin|cos paired in one tile (halves the
        # DVE instruction count of the recurrence)
        NS = J + 1
        pa_sl = [pp.tile([128, 2 * NQ], F16, name=f"pa{i}") for i in range(NS)]
        pb_sl = [pp.tile([128, 2 * NM], F16, name=f"pb{i}") for i in range(NS)]
        m2a = pp.tile([128, 2 * NQ], F16, name="m2a")
        m2b = pp.tile([128, 2 * NM], F16, name="m2b")

        with (
            tc.tile_pool(name="pb_ps", bufs=1, space="PSUM") as pbp,
            tc.tile_pool(name="o_ps", bufs=2, space="PSUM") as opp,
            tc.tile_pool(name="sc_ps", bufs=1, space="PSUM") as scp,
        ):
            def do_proj():
                # projections (fp32r: 1 cyc/row at free>=256)
                ps_a = pbp.tile([128, NQ], F32, name="ps_a")
                ps_b = pbp.tile([128, NM], F32, name="ps_b")
                for dt_ in range(KT):
                    nc.tensor.matmul(
                        ps_a[:], wa_sb[dt_][:], qT_sb[dt_][:],
                        start=(dt_ == 0), stop=(dt_ == KT - 1),
                    )
                for dt_ in range(KT):
                    nc.tensor.matmul(
                        ps_b[:], wb_sb[dt_][:], kT_sb[dt_][:],
                        start=(dt_ == 0), stop=(dt_ == KT - 1),
                    )
                return ps_a, ps_b

            ps_a, ps_b = do_proj()
            for r_ in range(reps):
                # ---- seeds: sin(d x), sin(d x / 2); cos via half-angle
                # (pair layout: [:, :W] = sin, [:, W:] = cos) ----
                pa = {1: pa_sl[1]}
                pb = {1: pb_sl[1]}
                nc.scalar.activation(pa[1][:, :NQ], ps_a[:], Sin, scale=DELTA,
                                     bias=zero_sb[:])
                ha = wp.tile([128, NQ], F16, name="ha")
                nc.scalar.activation(ha[:], ps_a[:], Sin, scale=DELTA / 2,
                                     bias=zero_sb[:])
                nc.scalar.activation(pb[1][:, :NM], ps_b[:], Sin, scale=DELTA,
                                     bias=sbias_sb[:])
                hb = wp.tile([128, NM], F16, name="hb")
                nc.scalar.activation(hb[:], ps_b[:], Sin, scale=DELTA / 2,
                                     bias=hbias_sb[:])
                h2a = wp.tile([128, NQ], F16, name="h2a")
                if sq_act:
                    nc.scalar.square(h2a[:], ha[:])
                else:
                    nc.vector.tensor_mul(h2a[:], ha[:], ha[:])
                nc.vector.tensor_scalar(pa[1][:, NQ:], h2a[:], -2.0, 1.0,
                                        MULT, ADD)
                nc.vector.tensor_scalar(m2a[:, :NQ], h2a[:], -4.0, 2.0,
                                        MULT, ADD)
                nc.vector.tensor_scalar(m2a[:, NQ:], h2a[:], -4.0, 2.0,
                                        MULT, ADD)
                h2b = wp.tile([128, NM], F16, name="h2b")
                if sq_act:
                    nc.scalar.square(h2b[:], hb[:])
                else:
                    nc.vector.tensor_mul(h2b[:], hb[:], hb[:])
                nc.vector.tensor_scalar(pb[1][:, NM:], h2b[:], -2.0, 1.0,
                                        MULT, ADD)
                nc.vector.tensor_scalar(m2b[:, :NM], h2b[:], -4.0, 2.0,
                                        MULT, ADD)
                nc.vector.tensor_scalar(m2b[:, NM:], h2b[:], -4.0, 2.0,
                                        MULT, ADD)

                def recur(j, p, p_sl, m2, w, tag):
                    pj = p_sl[j]
                    t = wp.tile([128, 2 * w], F16, name=f"r{tag}")
                    nc.vector.tensor_mul(t[:], m2[:], p[j - 1][:])
                    if j == 2:
                        # s0 = 0, c0 = 1
                        nc.vector.tensor_copy(pj[:, :w], t[:, :w])
                        nc.vector.tensor_scalar_add(pj[:, w:], t[:, w:], -1.0)
                    else:
                        nc.vector.tensor_sub(pj[:], t[:], p[j - 2][:])
                    p[j] = pj

                # scores: one PSUM bank per m-block, [m(128), n(512)]
                sc_t = [scp.tile([128, NQ], F32, name=f"sc{i}")
                        for i in range(MB)]
                exT = wp.tile([128, MB * NQ], F16, name="exT")

                for j in range(1, J + 1):
                    if j >= 2:
                        recur(j, pa, pa_sl, m2a, NQ, "a")
                        recur(j, pb, pb_sl, m2b, NM, "b")
                    fa = wp.tile([128, 2 * NQ], F16, name="fa")
                    if j > J - fold_act:
                        nc.scalar.mul(fa[:], pa[j][:], vbeta_sb[:, j - 1:j])
                    else:
                        nc.vector.tensor_scalar_mul(fa[:], pa[j][:],
                                                    vbeta_sb[:, j - 1:j])
                    for mb in range(MB):
                        bsl = slice(mb * 128, (mb + 1) * 128)
                        csl = slice(NM + mb * 128, NM + (mb + 1) * 128)
                        nc.tensor.matmul(
                            sc_t[mb][:], pb[j][:, csl], fa[:, :NQ],
                            start=(j == 1), stop=False,
                        )
                        nc.tensor.matmul(
                            sc_t[mb][:], pb[j][:, bsl], fa[:, NQ:],
                            start=False, stop=(j == J),
                        )

                # software-pipelined projections for the next rep: PE can
                # project while ACT runs this rep's exp
                if r_ + 1 < reps:
                    nps_a, nps_b = do_proj()
                # exp (scores bounded: no max-shift needed)
                for mb in range(MB):
                    nc.scalar.activation(
                        exT[:, mb * NQ:(mb + 1) * NQ], sc_t[mb][:], Exp)

                # out_u[n, d|1] = sum_m exT[m, n] [kf | 1]
                for nbk in range(NB):
                    po = opp.tile([128, 512], F32, name="po")
                    for mb in range(MB):
                        nc.tensor.matmul(
                            po[:, :D + 1],
                            exT[:, mb * NQ + nbk * 128: mb * NQ + nbk * 128 + 128],
                            kf_sb[mb][:],
                            start=(mb == 0), stop=(mb == MB - 1),
                        )
                    osb = sp.tile([128, D + 1], F32, name="osb")
                    nc.vector.tensor_copy(osb[:], po[:, :D + 1])
                    nc.sync.dma_start(
                        out_d[nbk * 128:(nbk + 1) * 128, :], osb[:])
                if r_ + 1 < reps:
                    ps_a, ps_b = nps_a, nps_b


def _in_maps(inputs):
    q = np.asarray(inputs["query"], dtype=np.float32)
    k = np.asarray(inputs["key"], dtype=np.float32)
    wa = np.ascontiguousarray(np.asarray(inputs["Wa_w"], dtype=np.float32))
    wb = np.ascontiguousarray(np.asarray(inputs["Wb_w"], dtype=np.float32))
    bias = (np.asarray(inputs["Wa_b"], dtype=np.float32)
            + np.asarray(inputs["Wb_b"], dtype=np.float32))
    v = np.asarray(inputs["v_w"], dtype=np.float32)
    sbias = (DELTA * bias).reshape(H, 1).astype(np.float32)
    hbias = (DELTA / 2 * bias).reshape(H, 1).astype(np.float32)
    vbeta = (v[:, None] * BETA[None, :]).astype(np.float32)
    maps = []
    for c in range(NCORES):
        b, nh, mh = c >> 2, (c >> 1) & 1, c & 1
        qs = q[b, nh * NQ:(nh + 1) * NQ, :]
        ks = k[b, mh * NM:(mh + 1) * NM, :]
        maps.append({
            "qT": np.ascontiguousarray(qs.T),
            "kT": np.ascontiguousarray(ks.T),
            "kf": np.ascontiguousarray(ks.astype(np.float16)),
            "wa": wa,
            "wb": wb,
            "sbias": sbias,
            "hbias": hbias,
            "vbeta": vbeta,
        })
    return maps


def _gather(results):
    out = np.empty((B, N, D), dtype=np.float32)
    for b in range(B):
        for nh in range(2):
            u0 = results[b * 4 + nh * 2 + 0]["out"]
            u1 = results[b * 4 + nh * 2 + 1]["out"]
            num = u0[:, :D] + u1[:, :D]
            den = u0[:, D] + u1[:, D]
            out[b, nh * NQ:(nh + 1) * NQ, :] = num / den[:, None]
    return out


_NC_CACHE = {}

BEST_OPTS = dict(fold_act=6, sq_act=True, wbufs=3)


def _get_nc(reps=1):
    if reps not in _NC_CACHE:
        _NC_CACHE[reps] = build_nc(reps, **BEST_OPTS)
    return _NC_CACHE[reps]


def kernel(**inputs):
    nc = _get_nc(1)
    res = bass_utils.run_bass_kernel_spmd(
        nc, _in_maps(inputs), core_ids=list(range(NCORES))
    )
    return _gather(res.results)
